# revision 1
# baseline (speedup 1.0000x reference)
"""Chamfer loss kernel for Trainium2 (8 NeuronCores, data-parallel over clouds).

Problem: N=8 clouds, subsample S=4096 points from each of two point sets,
compute per-cloud chamfer distance:
    loss[n] = mean_i min_j d(s1[n,i], s2[n,j]) + mean_j min_i d(...)

Strategy:
- Host: gather s1 = cloud1[:, idx1], s2 = cloud2[:, idx2] (cheap), then build
  matmul operands so that the PE array computes the full squared-distance
  matrix directly:  d_ij = sum_k A[k,i] * B[k,j]  with fp16 two-term splits
  of (-2*a), b, ||a||^2, ||b||^2 (K=13 rows, padded to 16). Each fp16*fp16
  product is exact in fp32; PSUM accumulates in fp32 => d is fp32-accurate
  (~1e-7 rel).
- Device (per core, one cloud): 32 i-tiles x (128 x 4096) distance tiles.
  PE: 8 matmuls (N=512) per i-tile -> PSUM. ACT: copy PSUM -> fp16 SBUF.
  DVE: tensor_reduce(min) over free axis = per-i min (direction a->b);
  tensor_tensor(min) accumulate = per-j running min (direction b->a, 2x fp16
  mode). Tail: DMA partition-shift + min tree, sums, ones-matmul partition
  reduction. Single scalar out per core.
- 8 cores run SPMD, one cloud each; host stacks the 8 scalars.
"""

import numpy as np

N_CLOUDS = 8
S = 4096  # subsampled points per cloud
K_ROWS = 16  # 13 used + 3 zero padding
P = 128  # partitions
NT = S // P  # 32 i-tiles
JW = 2048  # j-span width (one PSUM tile = 4 banks)
NSPAN = S // JW  # 2

_COMPILED = {}


def _build_bass(reps=1, b_engine="vector", tail_chunks=True):
    from contextlib import ExitStack

    from concourse import bacc
    import concourse.mybir as mybir
    from concourse.tile import TileContext

    fp16 = mybir.dt.float16
    fp32 = mybir.dt.float32
    MIN = mybir.AluOpType.min
    ADD = mybir.AluOpType.add
    X = mybir.AxisListType.X

    nc = bacc.Bacc("TRN2", target_bir_lowering=False)
    lhsT_d = nc.dram_tensor("lhsT", [K_ROWS, S], fp16, kind="ExternalInput")
    rhs_d = nc.dram_tensor("rhs", [K_ROWS, S], fp16, kind="ExternalInput")
    out_d = nc.dram_tensor("out", [1, 1], fp32, kind="ExternalOutput")

    with TileContext(nc) as tc, ExitStack() as ctx:
        const = ctx.enter_context(tc.tile_pool(name="const", bufs=1))
        psum = ctx.enter_context(tc.tile_pool(name="psum", bufs=2, space="PSUM"))
        dpool = ctx.enter_context(tc.tile_pool(name="dpool", bufs=4))
        small = ctx.enter_context(tc.tile_pool(name="small", bufs=1))

        def body():
            lhsT_s = const.tile([K_ROWS, S], fp16, tag="lhsT_s")
            rhs_s = const.tile([K_ROWS, S], fp16, tag="rhs_s")
            nc.gpsimd.dma_start(out=lhsT_s[:], in_=lhsT_d[:, :])
            nc.gpsimd.dma_start(out=rhs_s[:], in_=rhs_d[:, :])

            # ping-pong running column-min accumulators (direction b->a)
            bacc0 = const.tile([P, S], fp16, tag="bacc0")
            bacc1 = const.tile([P, S], fp16, tag="bacc1")
            rowmins = const.tile([P, NT], fp32, tag="rowmins")

            baccs = [bacc0, bacc1]
            for t in range(NT):
                src = baccs[t % 2]
                dst = baccs[(t + 1) % 2]
                d16 = dpool.tile([P, S], fp16, tag="d16")
                for s in range(NSPAN):
                    ps = psum.tile([P, JW], fp32, tag="ps")
                    for q in range(JW // 512):
                        j0 = s * JW + q * 512
                        nc.tensor.matmul(
                            ps[:, q * 512 : (q + 1) * 512],
                            lhsT_s[:, t * P : (t + 1) * P],
                            rhs_s[:, j0 : j0 + 512],
                            start=True,
                            stop=True,
                        )
                    sl = slice(s * JW, (s + 1) * JW)
                    nc.scalar.copy(d16[:, sl], ps[:])
                # direction b->a: one full-width running min over i-tiles
                # (first tile: plain copy at 4x instead of min with +inf)
                if t == 0:
                    nc.vector.tensor_copy(dst[:], d16[:])
                else:
                    nc.vector.tensor_tensor(dst[:], d16[:], src[:], op=MIN)
                # direction a->b: per-row min of this i-tile. tensor_reduce
                # only runs at 1x, so pre-fold with 2x-mode TT-min levels.
                m1 = dpool.tile([P, S // 2], fp16, tag="m1")
                nc.vector.tensor_tensor(
                    m1[:], d16[:, : S // 2], d16[:, S // 2 :], op=MIN
                )
                m2 = dpool.tile([P, S // 4], fp16, tag="m2")
                nc.vector.tensor_tensor(
                    m2[:], m1[:, : S // 4], m1[:, S // 4 :], op=MIN
                )
                m3 = dpool.tile([P, S // 8], fp16, tag="m3")
                nc.vector.tensor_tensor(
                    m3[:], m2[:, : S // 8], m2[:, S // 8 :], op=MIN
                )
                nc.vector.tensor_reduce(rowmins[:, t : t + 1], m3[:], axis=X, op=MIN)

            bfin = baccs[NT % 2]  # final accumulated column mins (128, S)

            # --- tail ---
            # a->b: sum of 4096 row mins
            rowsum = small.tile([P, 1], fp32, tag="rowsum")
            nc.vector.tensor_reduce(rowsum[:], rowmins[:], axis=X, op=ADD)

            # b->a: partition-halving min tree (128 -> 8) on (*, 4096) fp16.
            # DVE lanes cannot cross partitions, so shift the upper half down
            # with a SBUF->SBUF DMA first, then elementwise min at base 0.
            # Run the tree as NCH independent j-chunk pipelines with separate
            # tiles per (level, chunk): Tile's deps are tile-granular, so
            # separate tiles let chunk c's TT overlap chunk c+1's shift DMA
            # across all levels.
            NCH = 4 if tail_chunks else 1
            W = S // NCH
            coll8 = small.tile([8, S], fp16, tag="coll8")
            dma_engines = [nc.gpsimd, nc.sync, nc.scalar]
            for c in range(NCH):
                cur_c = bfin[:, c * W : (c + 1) * W]
                half = P // 2
                while half >= 8:
                    sh = small.tile([half, W], fp16, tag=f"sh{half}_{c}")
                    dma_engines[c % len(dma_engines)].dma_start(
                        out=sh[:], in_=cur_c[half : 2 * half, :])
                    if half == 8:
                        nx_ap = coll8[:, c * W : (c + 1) * W]
                    else:
                        nx = small.tile([half, W], fp16, tag=f"nx{half}_{c}")
                        nx_ap = nx[:]
                    nc.vector.tensor_tensor(
                        nx_ap, cur_c[0:half, :], sh[:], op=MIN
                    )
                    cur_c = nx_ap
                    half //= 2
            cur = coll8
            # cur: (8, 4096). Spread free axis over partitions so the rest of
            # the min tree runs wide: resh[r*8 + c, f] = cur[r, c*512 + f].
            # One DMA: both sides' linear walks match (r, c, f) <-> (r*8+c, f).
            resh = small.tile([64, 512], fp16, tag="resh")
            nc.gpsimd.dma_start(
                out=resh[:], in_=cur[:, :].rearrange("p (c f) -> p c f", f=512)
            )
            cur = resh
            half = 32
            lvl = 0
            while half >= 8:  # r-shifts: 32, 16, 8
                shifted = small.tile([half, 512], fp16, tag=f"shiftr{half}")
                dma_engines[lvl % len(dma_engines)].dma_start(
                    out=shifted[:], in_=cur[half : 2 * half, :]
                )
                nxt = small.tile([half, 512], fp16, tag=f"treer{half}")
                nc.vector.tensor_tensor(nxt[:], cur[0:half, :], shifted[:], op=MIN)
                cur = nxt
                half //= 2
                lvl += 1
            # cur: (8, 512) per-j column mins; sum them per partition
            bsum = small.tile([8, 1], fp32, tag="bsum")
            nc.vector.tensor_reduce(bsum[:], cur[:], axis=X, op=ADD)

            # partition sums on the PE, accumulated into one PSUM scalar
            ones = small.tile([P, 1], fp32, tag="ones")
            nc.vector.memset(ones[:], 1.0)
            acc = psum.tile([1, 1], fp32, tag="ps")
            nc.tensor.matmul(acc[:], rowsum[:], ones[:], start=True, stop=False)
            nc.tensor.matmul(acc[:], bsum[:], ones[:8, :], start=False, stop=True)
            res = small.tile([1, 1], fp32, tag="res")
            nc.scalar.mul(res[:], acc[:], 1.0 / S)
            nc.gpsimd.dma_start(out=out_d[:, :], in_=res[:])

        for _ in range(reps):
            body()

    nc.finalize()
    return nc


def _get_compiled():
    if "nc" not in _COMPILED:
        _COMPILED["nc"] = _build_bass()
    return _COMPILED["nc"]


def _split2(x):
    """fp16 two-term split: x ~= hi + lo with hi*anything exact in fp32."""
    hi = x.astype(np.float16)
    lo = (x - hi.astype(np.float32)).astype(np.float16)
    return hi, lo


def _build_operands(a, b):
    """a, b: (S, 3) fp32 -> A, B: (K_ROWS, S) fp16 with
    sum_k A[k,i]*B[k,j] = ||a_i||^2 + ||b_j||^2 - 2 a_i.b_j (fp32-accurate)."""
    A, B = [], []
    for c in range(3):
        ah, al = _split2(-2.0 * a[:, c])
        bh, bl = _split2(b[:, c])
        A += [ah, ah, al]
        B += [bh, bl, bh]
    sq1 = (a.astype(np.float64) ** 2).sum(1).astype(np.float32)
    sq2 = (b.astype(np.float64) ** 2).sum(1).astype(np.float32)
    ones = np.ones(a.shape[0], np.float16)
    s1h, s1l = _split2(sq1)
    s2h, s2l = _split2(sq2)
    A += [s1h, s1l, ones, ones]
    B += [ones, ones, s2h, s2l]
    z = np.zeros_like(ones)
    while len(A) < K_ROWS:
        A.append(z)
        B.append(z)
    return np.ascontiguousarray(np.stack(A)), np.ascontiguousarray(np.stack(B))


def _get_runner():
    """Build the sharded jitted executable once and cache it; re-tracing the
    PJRT wrapper per call costs ~250 ms otherwise."""
    if "runner" in _COMPILED:
        return _COMPILED["runner"]
    import jax
    from jax.sharding import Mesh, PartitionSpec
    import warnings
    with warnings.catch_warnings():
        warnings.simplefilter("ignore")
        from jax.experimental.shard_map import shard_map
    import concourse.mybir as mybir
    from concourse import bass2jax

    nc = _get_compiled()
    bass2jax.install_neuronx_cc_hook()
    partition_name = nc.partition_id_tensor.name if nc.partition_id_tensor else None
    in_names, out_names, out_avals, zero_outs = [], [], [], []
    for alloc in nc.m.functions[0].allocations:
        if not isinstance(alloc, mybir.MemoryLocationSet):
            continue
        name = alloc.memorylocations[0].name
        if alloc.kind == "ExternalInput":
            if name != partition_name:
                in_names.append(name)
        elif alloc.kind == "ExternalOutput":
            shape = tuple(alloc.tensor_shape)
            dtype = mybir.dt.np(alloc.dtype)
            out_avals.append(jax.core.ShapedArray(shape, dtype))
            out_names.append(name)
            zero_outs.append(np.zeros(shape, dtype))
    n_params = len(in_names)
    all_in = list(in_names) + list(out_names)
    if partition_name is not None:
        all_in.append(partition_name)

    def _body(*args):
        operands = list(args)
        if partition_name is not None:
            operands.append(bass2jax.partition_id_tensor())
        outs = bass2jax._bass_exec_p.bind(
            *operands,
            out_avals=tuple(out_avals),
            in_names=tuple(all_in),
            out_names=tuple(out_names),
            lowering_input_output_aliases=(),
            sim_require_finite=True,
            sim_require_nnan=True,
            nc=nc,
        )
        return tuple(outs)

    devices = jax.devices()[:N_CLOUDS]
    mesh = Mesh(np.asarray(devices), ("core",))
    in_specs = (PartitionSpec("core"),) * (n_params + len(out_avals))
    out_specs = (PartitionSpec("core"),) * len(out_avals)
    fn = jax.jit(
        shard_map(_body, mesh=mesh, in_specs=in_specs, out_specs=out_specs,
                  check_rep=False),
        keep_unused=True,
    )
    runner = (fn, in_names, zero_outs)
    _COMPILED["runner"] = runner
    return runner


def kernel(cloud1, cloud2, idx1, idx2, num_samples):

    cloud1 = np.asarray(cloud1, dtype=np.float32)
    cloud2 = np.asarray(cloud2, dtype=np.float32)
    i1 = np.asarray(idx1).astype(np.int64)
    i2 = np.asarray(idx2).astype(np.int64)
    ns = int(np.asarray(num_samples))
    assert ns == S and i1.shape[0] == S and i2.shape[0] == S
    assert cloud1.shape[0] == N_CLOUDS

    s1 = cloud1[:, i1, :]  # (8, S, 3)
    s2 = cloud2[:, i2, :]

    # build all 8 cores' operands vectorized: (8, K_ROWS, S) each
    A, B = [], []
    for c in range(3):
        ah, al = _split2(-2.0 * s1[:, :, c])
        bh, bl = _split2(s2[:, :, c])
        A += [ah, ah, al]
        B += [bh, bl, bh]
    sq1 = (s1.astype(np.float64) ** 2).sum(-1).astype(np.float32)
    sq2 = (s2.astype(np.float64) ** 2).sum(-1).astype(np.float32)
    ones = np.ones((N_CLOUDS, S), np.float16)
    s1h, s1l = _split2(sq1)
    s2h, s2l = _split2(sq2)
    A += [s1h, s1l, ones, ones]
    B += [ones, ones, s2h, s2l]
    z = np.zeros_like(ones)
    while len(A) < K_ROWS:
        A.append(z)
        B.append(z)
    Aall = np.ascontiguousarray(np.stack(A, axis=1))  # (8, K_ROWS, S)
    Ball = np.ascontiguousarray(np.stack(B, axis=1))
    by_name = {"lhsT": Aall.reshape(-1, S), "rhs": Ball.reshape(-1, S)}

    fn, in_names, zero_outs = _get_runner()
    concat_in = [by_name[nm] for nm in in_names]
    concat_zeros = [
        np.zeros((N_CLOUDS * z.shape[0], *z.shape[1:]), z.dtype) for z in zero_outs
    ]
    out_arrs = fn(*concat_in, *concat_zeros)
    out = np.asarray(out_arrs[0]).reshape(N_CLOUDS).astype(np.float32)
    return out

